# revision 27
# baseline (speedup 1.0000x reference)
"""Causal self-attention (B=4, T=2048, D=1024, H=16) on 8 TRN2 NeuronCores.

Sharding: core c handles batch b = c//2 and head-group g = c%2 (8 of 16 heads).
Each core computes qkv for its heads, causal attention, and a partial
projection out_partial = y_local @ w_proj[rows_g]; the host sums the two
head-group partials per batch.

Dataflow on device (per core):
  - x^T [D, T] resident in SBUF (fp32r).  qk^T m-tiles [128, T] = w_slice.T @ x^T
    give Q^T/K^T per head-pair directly (no on-device transposes).
  - S^T[keys, q] = K @ Q^T via two K=64 matmuls packed into the PE array with
    tile_position row packing (head A rows 0-63, head B rows 64-127).
  - P^T = exp(S^T / 8) on ScalarE (no max-subtraction: |S| is O(5) for this
    distribution, exp stays in fp32 range), causal mask applied as a 0/1
    multiply on the 4 diagonal key-blocks only.
  - O^T[65, q] accumulates V_aug^T @ P^T in PSUM where V_aug = [V | 1]; row 64
    is the softmax denominator for free.
  - normalize: denom -> reciprocal -> partition_broadcast -> multiply.
  - proj consumes y^T [512, T] (bf16) against w_proj rows (bf16).

Precision: score path fp32r (~2e-4), value path bf16 (~3e-3) — well inside a
2e-2 gate while running the PE at full (1 cycle/row) speed.
"""

import tempfile

import numpy as np
import ml_dtypes

import concourse.bass as bass
import concourse.mybir as mybir
import concourse.tile as tile
from concourse import bacc
from concourse import bass_utils

# Problem constants (hardcoded per contract).
B, T, D, H = 4, 2048, 1024, 16
HD = D // H  # 64
N_CORES = 8
P = 128
DL = 512  # local head dims per core (8 heads x 64)
NKT = T // P  # 16 key tiles
NQB = T // 512  # 4 query blocks
NKD = D // P  # 8 contraction tiles over D

F32 = mybir.dt.float32
F32R = mybir.dt.float32r
BF16 = mybir.dt.bfloat16
EXP = mybir.ActivationFunctionType.Exp

# Set by test harness to capture a profile; harness default leaves it off.
PROFILE = False
TRACE_DIR = "/tmp/attn_neff"

_CACHE = {}


def _build_program():
    nc = bacc.Bacc("TRN2", target_bir_lowering=False, debug=False,
                   num_devices=N_CORES)

    xT = nc.dram_tensor("xT", [D, T], F32R, kind="ExternalInput").ap()
    wqk = nc.dram_tensor("wqk", [D, D], F32R, kind="ExternalInput").ap()
    wv = nc.dram_tensor("wv", [D, DL], F32R, kind="ExternalInput").ap()
    wp = nc.dram_tensor("wp", [DL, D], BF16, kind="ExternalInput").ap()
    masks = nc.dram_tensor("masks", [P, P], BF16, kind="ExternalInput").ap()
    out = nc.dram_tensor("out", [T, D], F32, kind="ExternalOutput").ap()

    with tile.TileContext(nc) as tc:
        with (
            tc.tile_pool(name="singles", bufs=1) as singles,
            tc.tile_pool(name="vdram", bufs=1, space="DRAM") as vdram,
            tc.tile_pool(name="qk", bufs=2) as qk_pool,
            tc.tile_pool(name="vp", bufs=2) as vp_pool,
            tc.tile_pool(name="pt", bufs=3) as pt_pool,
            tc.tile_pool(name="nrm", bufs=2) as nrm_pool,
            tc.tile_pool(name="ps_mm", bufs=2, space="PSUM") as ps_mm,
            tc.tile_pool(name="ps_s", bufs=2, space="PSUM") as ps_s,
            tc.tile_pool(name="ps_o", bufs=1, space="PSUM") as ps_o,
        ):
            # ---- resident inputs ----
            # wv first (small, gates the v-stage), then xT in per-ktile chunks
            # so QKV accumulation chains start as soon as each chunk lands.
            wv_sb = singles.tile([P, NKD * DL], F32R)
            nc.sync.dma_start(
                wv_sb[:].rearrange("p (k m) -> p k m", k=NKD),
                wv.rearrange("(k p) m -> p k m", p=P),
            )
            xT_sb = singles.tile([P, NKD * T], F32R)  # [128, 8*2048]
            for kd in range(NKD):
                nc.sync.dma_start(
                    xT_sb[:, kd * T:(kd + 1) * T],
                    xT[kd * P:(kd + 1) * P, :],
                )
            masks_sb = singles.tile([P, P], BF16)
            nc.sync.dma_start(masks_sb[:], masks)
            yT_sb = singles.tile([P, 4 * T], BF16)
            wp_sb = singles.tile([P, 4 * D], BF16)

            # pair 0's wqk slice loads ahead of the bulk xT stream so its
            # qT/kT chains can start chasing the xT chunks immediately
            wqk_p0 = qk_pool.tile([P, NKD * 256], F32R, name="wqkp0", tag="wqk")
            nc.gpsimd.dma_start(
                wqk_p0[:].rearrange("p (k m) -> p k m", k=NKD)[:, :, 0:128],
                wqk.rearrange("(k p) m -> p k m", p=P)[:, :, 0:P],
            )
            nc.gpsimd.dma_start(
                wqk_p0[:].rearrange("p (k m) -> p k m", k=NKD)[:, :, 128:256],
                wqk.rearrange("(k p) m -> p k m", p=P)[:, :, DL: DL + P],
            )

            v_dram = vdram.tile([P, NKT, 8 * 65], BF16)

            # ---- v stage: v_aug[t] = [x @ w_v | 1] for all 8 local heads ----
            for t in range(NKT):
                mm = ps_mm.tile([P, DL], F32, name=f"mmv{t}", tag="mm")
                for kd in range(NKD):
                    nc.tensor.matmul(
                        mm[:],
                        xT_sb[:, kd * T + t * P: kd * T + (t + 1) * P],
                        wv_sb[:, kd * DL: (kd + 1) * DL],
                        start=(kd == 0), stop=(kd == NKD - 1),
                    )
                vstage = vp_pool.tile([P, 8 * 65], BF16, name=f"vst{t}", tag="vst")
                nc.vector.memset(vstage[:], 1.0)
                nc.vector.tensor_copy(
                    vstage[:].rearrange("p (h e) -> p h e", e=65)[:, :, 0:64],
                    mm[:].rearrange("p (h e) -> p h e", e=64),
                )
                nc.sync.dma_start(v_dram[:, t, :], vstage[:])

            # ---- per head-pair ----
            for pr in range(4):
                hA, hB = 2 * pr, 2 * pr + 1

                # wqk slice for this pair: q m-tile (cols pr*128) and k m-tile
                # (cols 512 + pr*128), interleaved as [128, kd, 256].
                if pr == 0:
                    wqk_p = wqk_p0
                else:
                    wqk_p = qk_pool.tile([P, NKD * 256], F32R, name=f"wqkp{pr}",
                                         tag="wqk")
                    nc.sync.dma_start(
                        wqk_p[:].rearrange("p (k m) -> p k m", k=NKD)[:, :, 0:128],
                        wqk.rearrange("(k p) m -> p k m", p=P)[
                            :, :, pr * P:(pr + 1) * P],
                    )
                    nc.sync.dma_start(
                        wqk_p[:].rearrange("p (k m) -> p k m", k=NKD)[:, :, 128:256],
                        wqk.rearrange("(k p) m -> p k m", p=P)[
                            :, :, DL + pr * P: DL + (pr + 1) * P],
                    )

                qT = qk_pool.tile([P, T], F32R, name=f"qT{pr}", tag="qT")
                kT = qk_pool.tile([P, T], F32R, name=f"kT{pr}", tag="kT")
                for dst, mo in ((qT, 0), (kT, 128)):
                    for n in range(4):
                        mm = ps_mm.tile([P, 512], F32, name=f"mmqk{pr}{mo}{n}", tag="mm")
                        for kd in range(NKD):
                            nc.tensor.matmul(
                                mm[:],
                                wqk_p[:, kd * 256 + mo: kd * 256 + mo + 128],
                                xT_sb[:, kd * T + n * 512: kd * T + (n + 1) * 512],
                                start=(kd == 0), stop=(kd == NKD - 1),
                            )
                        nc.vector.tensor_copy(dst[:, n * 512:(n + 1) * 512], mm[:])

                # v_aug for this pair: [128, kb, 130]
                v_p = vp_pool.tile([P, NKT * 130], BF16, name=f"vp{pr}", tag="vp")
                nc.sync.dma_start(
                    v_p[:].rearrange("p (t e) -> p t e", t=NKT),
                    v_dram[:, :, hA * 65: hA * 65 + 130],
                )

                if pr == 3:
                    nc.sync.dma_start(
                        wp_sb[:].rearrange("p (k m) -> p k m", k=4),
                        wp.rearrange("(k p) m -> p k m", p=P),
                    )

                # largest query block first: gives the S->exp->O pipeline its
                # longest runway right after the pair's qT/kT land, and ends
                # each pair on the shortest block so the next pair (or the
                # projection tail) starts sooner.
                for qb in reversed(range(NQB)):
                    nkb = 4 * (qb + 1)
                    oA = ps_o.tile([65, 512], F32, name=f"oA{pr}{qb}", tag="oA")
                    oB = ps_o.tile([65, 512], F32, name=f"oB{pr}{qb}", tag="oB")
                    for kb in range(nkb):
                        j = kb - (nkb - 4)
                        # visible queries for this key block start at q_off
                        # (diagonal blocks j>=1 see only the last 512-j*128
                        # queries of the block; keep f32r N>=256 for S).
                        q_off = max(0, j) * P
                        s_off = min(q_off, 256)
                        qn = 512 - q_off
                        s = ps_s.tile([P, 1024], F32, name=f"s{pr}{qb}{kb}", tag="s")
                        nc.tensor.matmul(
                            s[:, s_off:512],
                            kT[0:64, kb * P:(kb + 1) * P],
                            qT[0:64, qb * 512 + s_off: (qb + 1) * 512],
                            start=True, stop=True, tile_position=(0, 0),
                        )
                        nc.tensor.matmul(
                            s[:, 512 + s_off:1024],
                            kT[64:128, kb * P:(kb + 1) * P],
                            qT[64:128, qb * 512 + s_off: (qb + 1) * 512],
                            start=True, stop=True, tile_position=(64, 0),
                        )
                        pt = pt_pool.tile([P, 1024], BF16, name=f"pt{pr}{qb}{kb}",
                                          tag="pt")
                        if q_off == 0:
                            nc.scalar.activation(pt[:], s[:], EXP, scale=0.125)
                        else:
                            nc.scalar.activation(pt[:, q_off:512],
                                                 s[:, q_off:512], EXP, scale=0.125)
                            nc.scalar.activation(pt[:, 512 + q_off:1024],
                                                 s[:, 512 + q_off:1024], EXP,
                                                 scale=0.125)
                        if j >= 0:
                            # triangle mask on the single partially-visible
                            # 128-query sub-block of each head
                            nc.vector.tensor_mul(
                                pt[:, q_off:q_off + P], pt[:, q_off:q_off + P],
                                masks_sb[:])
                            nc.vector.tensor_mul(
                                pt[:, 512 + q_off:512 + q_off + P],
                                pt[:, 512 + q_off:512 + q_off + P], masks_sb[:])
                        nc.tensor.matmul(
                            oA[:, q_off:512], v_p[:, kb * 130: kb * 130 + 65],
                            pt[:, q_off:512],
                            start=(kb == 0), stop=(kb == nkb - 1),
                        )
                        nc.tensor.matmul(
                            oB[:, q_off:512],
                            v_p[:, kb * 130 + 65: kb * 130 + 130],
                            pt[:, 512 + q_off:1024],
                            start=(kb == 0), stop=(kb == nkb - 1),
                        )
                    # normalize: row 64 of each O psum is the denominator.
                    # Copy O psums to SBUF first so the banks free immediately
                    # (the normalize tail then runs off the PE critical path).
                    unA = nrm_pool.tile([65, 512], F32, name=f"unA{pr}{qb}", tag="unA")
                    unB = nrm_pool.tile([65, 512], F32, name=f"unB{pr}{qb}", tag="unB")
                    nc.vector.tensor_copy(unA[:], oA[:])
                    nc.vector.tensor_copy(unB[:], oB[:])
                    # reciprocal is an iterative 8-cyc/elem op: respread the
                    # [1, 512] rows across 128 partitions so it takes ~170ns
                    # instead of 3.3us, then shift to p0 for partition_broadcast
                    # (which reads tensor partition 0 on HW).
                    den_p = nrm_pool.tile([P, 8], F32, name=f"denp{pr}{qb}", tag="denp")
                    nc.gpsimd.dma_start(den_p[:, 0:4], unA[64:65, :])
                    nc.gpsimd.dma_start(den_p[:, 4:8], unB[64:65, :])
                    nc.vector.reciprocal(den_p[:], den_p[:])
                    rec0 = nrm_pool.tile([1, 1024], F32, name=f"rec0{pr}{qb}",
                                         tag="rec0")
                    nc.gpsimd.dma_start(rec0[0:1, 0:512], den_p[:, 0:4])
                    nc.gpsimd.dma_start(rec0[0:1, 512:1024], den_p[:, 4:8])
                    bcA = nrm_pool.tile([64, 512], F32, name=f"bcA{pr}{qb}", tag="bcA")
                    bcB = nrm_pool.tile([64, 512], F32, name=f"bcB{pr}{qb}", tag="bcB")
                    nc.gpsimd.partition_broadcast(bcA[:], rec0[0:1, 0:512])
                    nc.gpsimd.partition_broadcast(bcB[:], rec0[0:1, 512:1024])
                    nc.vector.tensor_mul(
                        yT_sb[0:64, pr * T + qb * 512: pr * T + (qb + 1) * 512],
                        unA[0:64, :], bcA[:])
                    stB = nrm_pool.tile([64, 512], BF16, name=f"stB{pr}{qb}", tag="stB")
                    nc.vector.tensor_mul(stB[:], unB[0:64, :], bcB[:])
                    nc.sync.dma_start(
                        yT_sb[64:128, pr * T + qb * 512: pr * T + (qb + 1) * 512],
                        stB[:])

                    # ---- proj, interleaved: once pair 3 normalizes query
                    # block qb, the output rows t in [4qb, 4qb+4) have all four
                    # pairs' yT ready — emit them here so the projection hides
                    # under pair 3's remaining (ACT-bound) attention instead of
                    # serializing at the kernel tail.
                    if pr == 3:
                        for t in range(4 * qb, 4 * qb + 4):
                            for n in range(2):
                                mm = ps_mm.tile([P, 512], F32, name=f"mmo{t}{n}",
                                                tag="mm")
                                for kp in range(4):
                                    nc.tensor.matmul(
                                        mm[:],
                                        yT_sb[:, kp * T + t * P: kp * T + (t + 1) * P],
                                        wp_sb[:, kp * D + n * 512:
                                              kp * D + (n + 1) * 512],
                                        start=(kp == 0), stop=(kp == 3),
                                    )
                                ost = nrm_pool.tile([P, 512], F32,
                                                    name=f"ost{t}{n}", tag="ost")
                                nc.scalar.activation(
                                    ost[:], mm[:],
                                    mybir.ActivationFunctionType.Copy)
                                nc.sync.dma_start(
                                    out[t * P:(t + 1) * P, n * 512:(n + 1) * 512],
                                    ost[:])

    nc.compile()
    return nc


def _host_masks():
    k = np.arange(P)[:, None]
    q = np.arange(P)[None, :]
    return (k <= q).astype(ml_dtypes.bfloat16)


def _in_maps(x, w_qkv, w_proj):
    masks = _host_masks()
    maps = []
    for c in range(N_CORES):
        b, g = c // 2, c % 2
        xTc = np.ascontiguousarray(x[b].T.astype(np.float32))
        wqk = np.concatenate(
            [w_qkv[:, g * DL:(g + 1) * DL], w_qkv[:, D + g * DL: D + (g + 1) * DL]],
            axis=1).astype(np.float32)
        wvc = np.ascontiguousarray(w_qkv[:, 2 * D + g * DL: 2 * D + (g + 1) * DL]
                                   .astype(np.float32))
        wpc = np.ascontiguousarray(w_proj[g * DL:(g + 1) * DL, :]).astype(
            ml_dtypes.bfloat16)
        maps.append({"xT": xTc, "wqk": wqk, "wv": wvc, "wp": wpc, "masks": masks})
    return maps


def kernel(x, w_qkv, w_proj):
    x = np.asarray(x, dtype=np.float32)
    w_qkv = np.asarray(w_qkv, dtype=np.float32)
    w_proj = np.asarray(w_proj, dtype=np.float32)

    if "nc" not in _CACHE:
        _CACHE["nc"] = _build_program()
    nc = _CACHE["nc"]

    tmpdir = tempfile.mkdtemp(prefix="attn_neff_") if PROFILE else None
    res = bass_utils.run_bass_kernel_spmd(
        nc, _in_maps(x, w_qkv, w_proj), core_ids=list(range(N_CORES)),
        trace=PROFILE, tmpdir=tmpdir,
    )
    _CACHE["last_result"] = res

    out = np.empty((B, T, D), dtype=np.float32)
    for b in range(B):
        out[b] = res.results[2 * b]["out"] + res.results[2 * b + 1]["out"]
    return out


# revision 28
# speedup vs baseline: 1.0513x; 1.0513x over previous
"""Causal self-attention (B=4, T=2048, D=1024, H=16) on 8 TRN2 NeuronCores.

Sharding: core c handles batch b = c//2 and head-group g = c%2 (8 of 16 heads).
Each core computes qkv for its heads, causal attention, and a partial
projection out_partial = y_local @ w_proj[rows_g]; the host sums the two
head-group partials per batch.

Dataflow on device (per core):
  - x^T [D, T] resident in SBUF (fp32r).  qk^T m-tiles [128, T] = w_slice.T @ x^T
    give Q^T/K^T per head-pair directly (no on-device transposes).
  - S^T[keys, q] = K @ Q^T via two K=64 matmuls packed into the PE array with
    tile_position row packing (head A rows 0-63, head B rows 64-127).
  - P^T = exp(S^T / 8) on ScalarE (no max-subtraction: |S| is O(5) for this
    distribution, exp stays in fp32 range), causal mask applied as a 0/1
    multiply on the 4 diagonal key-blocks only.
  - O^T[65, q] accumulates V_aug^T @ P^T in PSUM where V_aug = [V | 1]; row 64
    is the softmax denominator for free.
  - normalize: denom -> reciprocal -> partition_broadcast -> multiply.
  - proj consumes y^T [512, T] (bf16) against w_proj rows (bf16).

Precision: score path fp32r (~2e-4), value path bf16 (~3e-3) — well inside a
2e-2 gate while running the PE at full (1 cycle/row) speed.
"""

import tempfile

import numpy as np
import ml_dtypes

import concourse.bass as bass
import concourse.mybir as mybir
import concourse.tile as tile
from concourse import bacc
from concourse import bass_utils

# Problem constants (hardcoded per contract).
B, T, D, H = 4, 2048, 1024, 16
HD = D // H  # 64
N_CORES = 8
P = 128
DL = 512  # local head dims per core (8 heads x 64)
NKT = T // P  # 16 key tiles
NQB = T // 512  # 4 query blocks
NKD = D // P  # 8 contraction tiles over D

F32 = mybir.dt.float32
F32R = mybir.dt.float32r
BF16 = mybir.dt.bfloat16
EXP = mybir.ActivationFunctionType.Exp

# Set by test harness to capture a profile; harness default leaves it off.
PROFILE = False
TRACE_DIR = "/tmp/attn_neff"

_CACHE = {}


def _build_program():
    nc = bacc.Bacc("TRN2", target_bir_lowering=False, debug=False,
                   num_devices=N_CORES)

    xT = nc.dram_tensor("xT", [D, T], F32R, kind="ExternalInput").ap()
    wqk = nc.dram_tensor("wqk", [D, D], F32R, kind="ExternalInput").ap()
    wv = nc.dram_tensor("wv", [D, DL], F32R, kind="ExternalInput").ap()
    wp = nc.dram_tensor("wp", [DL, D], BF16, kind="ExternalInput").ap()
    masks = nc.dram_tensor("masks", [P, P], BF16, kind="ExternalInput").ap()
    out = nc.dram_tensor("out", [T, D], F32, kind="ExternalOutput").ap()

    with tile.TileContext(nc) as tc:
        with (
            tc.tile_pool(name="singles", bufs=1) as singles,
            tc.tile_pool(name="vdram", bufs=1, space="DRAM") as vdram,
            tc.tile_pool(name="qk", bufs=2) as qk_pool,
            tc.tile_pool(name="vp", bufs=2) as vp_pool,
            tc.tile_pool(name="pt", bufs=3) as pt_pool,
            tc.tile_pool(name="nrm", bufs=2) as nrm_pool,
            tc.tile_pool(name="ps_mm", bufs=2, space="PSUM") as ps_mm,
            tc.tile_pool(name="ps_s", bufs=2, space="PSUM") as ps_s,
            tc.tile_pool(name="ps_o", bufs=1, space="PSUM") as ps_o,
        ):
            # ---- resident inputs ----
            # wv first (small, gates the v-stage), then xT in per-ktile chunks
            # so QKV accumulation chains start as soon as each chunk lands.
            wv_sb = singles.tile([P, NKD * DL], F32R)
            nc.sync.dma_start(
                wv_sb[:].rearrange("p (k m) -> p k m", k=NKD),
                wv.rearrange("(k p) m -> p k m", p=P),
            )
            xT_sb = singles.tile([P, NKD * T], F32R)  # [128, 8*2048]
            for kd in range(NKD):
                nc.sync.dma_start(
                    xT_sb[:, kd * T:(kd + 1) * T],
                    xT[kd * P:(kd + 1) * P, :],
                )
            masks_sb = singles.tile([P, P], BF16)
            nc.sync.dma_start(masks_sb[:], masks)
            yT_sb = singles.tile([P, 4 * T], BF16)
            wp_sb = singles.tile([P, 4 * D], BF16)

            v_dram = vdram.tile([P, NKT, 8 * 65], BF16)

            # ---- v stage: v_aug[t] = [x @ w_v | 1] for all 8 local heads ----
            for t in range(NKT):
                mm = ps_mm.tile([P, DL], F32, name=f"mmv{t}", tag="mm")
                for kd in range(NKD):
                    nc.tensor.matmul(
                        mm[:],
                        xT_sb[:, kd * T + t * P: kd * T + (t + 1) * P],
                        wv_sb[:, kd * DL: (kd + 1) * DL],
                        start=(kd == 0), stop=(kd == NKD - 1),
                    )
                vstage = vp_pool.tile([P, 8 * 65], BF16, name=f"vst{t}", tag="vst")
                nc.vector.memset(vstage[:], 1.0)
                nc.vector.tensor_copy(
                    vstage[:].rearrange("p (h e) -> p h e", e=65)[:, :, 0:64],
                    mm[:].rearrange("p (h e) -> p h e", e=64),
                )
                nc.sync.dma_start(v_dram[:, t, :], vstage[:])

            # ---- per head-pair ----
            for pr in range(4):
                hA, hB = 2 * pr, 2 * pr + 1

                # wqk slice for this pair: q m-tile (cols pr*128) and k m-tile
                # (cols 512 + pr*128), interleaved as [128, kd, 256].
                wqk_p = qk_pool.tile([P, NKD * 256], F32R, name=f"wqkp{pr}", tag="wqk")
                nc.sync.dma_start(
                    wqk_p[:].rearrange("p (k m) -> p k m", k=NKD)[:, :, 0:128],
                    wqk.rearrange("(k p) m -> p k m", p=P)[:, :, pr * P:(pr + 1) * P],
                )
                nc.sync.dma_start(
                    wqk_p[:].rearrange("p (k m) -> p k m", k=NKD)[:, :, 128:256],
                    wqk.rearrange("(k p) m -> p k m", p=P)[
                        :, :, DL + pr * P: DL + (pr + 1) * P],
                )

                qT = qk_pool.tile([P, T], F32R, name=f"qT{pr}", tag="qT")
                kT = qk_pool.tile([P, T], F32R, name=f"kT{pr}", tag="kT")
                for dst, mo in ((qT, 0), (kT, 128)):
                    for n in range(4):
                        mm = ps_mm.tile([P, 512], F32, name=f"mmqk{pr}{mo}{n}", tag="mm")
                        for kd in range(NKD):
                            nc.tensor.matmul(
                                mm[:],
                                wqk_p[:, kd * 256 + mo: kd * 256 + mo + 128],
                                xT_sb[:, kd * T + n * 512: kd * T + (n + 1) * 512],
                                start=(kd == 0), stop=(kd == NKD - 1),
                            )
                        nc.vector.tensor_copy(dst[:, n * 512:(n + 1) * 512], mm[:])

                # v_aug for this pair: [128, kb, 130]
                v_p = vp_pool.tile([P, NKT * 130], BF16, name=f"vp{pr}", tag="vp")
                nc.sync.dma_start(
                    v_p[:].rearrange("p (t e) -> p t e", t=NKT),
                    v_dram[:, :, hA * 65: hA * 65 + 130],
                )

                if pr == 3:
                    nc.sync.dma_start(
                        wp_sb[:].rearrange("p (k m) -> p k m", k=4),
                        wp.rearrange("(k p) m -> p k m", p=P),
                    )

                # largest query block first: gives the S->exp->O pipeline its
                # longest runway right after the pair's qT/kT land, and ends
                # each pair on the shortest block so the next pair (or the
                # projection tail) starts sooner.
                for qb in reversed(range(NQB)):
                    nkb = 4 * (qb + 1)
                    oA = ps_o.tile([65, 512], F32, name=f"oA{pr}{qb}", tag="oA")
                    oB = ps_o.tile([65, 512], F32, name=f"oB{pr}{qb}", tag="oB")
                    for kb in range(nkb):
                        j = kb - (nkb - 4)
                        # visible queries for this key block start at q_off
                        # (diagonal blocks j>=1 see only the last 512-j*128
                        # queries of the block; keep f32r N>=256 for S).
                        q_off = max(0, j) * P
                        s_off = min(q_off, 256)
                        qn = 512 - q_off
                        s = ps_s.tile([P, 1024], F32, name=f"s{pr}{qb}{kb}", tag="s")
                        nc.tensor.matmul(
                            s[:, s_off:512],
                            kT[0:64, kb * P:(kb + 1) * P],
                            qT[0:64, qb * 512 + s_off: (qb + 1) * 512],
                            start=True, stop=True, tile_position=(0, 0),
                        )
                        nc.tensor.matmul(
                            s[:, 512 + s_off:1024],
                            kT[64:128, kb * P:(kb + 1) * P],
                            qT[64:128, qb * 512 + s_off: (qb + 1) * 512],
                            start=True, stop=True, tile_position=(64, 0),
                        )
                        pt = pt_pool.tile([P, 1024], BF16, name=f"pt{pr}{qb}{kb}",
                                          tag="pt")
                        if q_off == 0:
                            nc.scalar.activation(pt[:], s[:], EXP, scale=0.125)
                        else:
                            nc.scalar.activation(pt[:, q_off:512],
                                                 s[:, q_off:512], EXP, scale=0.125)
                            nc.scalar.activation(pt[:, 512 + q_off:1024],
                                                 s[:, 512 + q_off:1024], EXP,
                                                 scale=0.125)
                        if j >= 0:
                            # triangle mask on the single partially-visible
                            # 128-query sub-block of each head
                            nc.vector.tensor_mul(
                                pt[:, q_off:q_off + P], pt[:, q_off:q_off + P],
                                masks_sb[:])
                            nc.vector.tensor_mul(
                                pt[:, 512 + q_off:512 + q_off + P],
                                pt[:, 512 + q_off:512 + q_off + P], masks_sb[:])
                        nc.tensor.matmul(
                            oA[:, q_off:512], v_p[:, kb * 130: kb * 130 + 65],
                            pt[:, q_off:512],
                            start=(kb == 0), stop=(kb == nkb - 1),
                        )
                        nc.tensor.matmul(
                            oB[:, q_off:512],
                            v_p[:, kb * 130 + 65: kb * 130 + 130],
                            pt[:, 512 + q_off:1024],
                            start=(kb == 0), stop=(kb == nkb - 1),
                        )
                    # normalize: row 64 of each O psum is the denominator.
                    # Copy O psums to SBUF first so the banks free immediately
                    # (the normalize tail then runs off the PE critical path).
                    unA = nrm_pool.tile([65, 512], F32, name=f"unA{pr}{qb}", tag="unA")
                    unB = nrm_pool.tile([65, 512], F32, name=f"unB{pr}{qb}", tag="unB")
                    nc.vector.tensor_copy(unA[:], oA[:])
                    nc.vector.tensor_copy(unB[:], oB[:])
                    # reciprocal is an iterative 8-cyc/elem op: respread the
                    # [1, 512] rows across 128 partitions so it takes ~170ns
                    # instead of 3.3us, then shift to p0 for partition_broadcast
                    # (which reads tensor partition 0 on HW).
                    den_p = nrm_pool.tile([P, 8], F32, name=f"denp{pr}{qb}", tag="denp")
                    nc.sync.dma_start(den_p[:, 0:4], unA[64:65, :])
                    nc.sync.dma_start(den_p[:, 4:8], unB[64:65, :])
                    nc.vector.reciprocal(den_p[:], den_p[:])
                    rec0 = nrm_pool.tile([1, 1024], F32, name=f"rec0{pr}{qb}",
                                         tag="rec0")
                    nc.sync.dma_start(rec0[0:1, 0:512], den_p[:, 0:4])
                    nc.sync.dma_start(rec0[0:1, 512:1024], den_p[:, 4:8])
                    bcA = nrm_pool.tile([64, 512], F32, name=f"bcA{pr}{qb}", tag="bcA")
                    bcB = nrm_pool.tile([64, 512], F32, name=f"bcB{pr}{qb}", tag="bcB")
                    nc.gpsimd.partition_broadcast(bcA[:], rec0[0:1, 0:512])
                    nc.gpsimd.partition_broadcast(bcB[:], rec0[0:1, 512:1024])
                    nc.vector.tensor_mul(
                        yT_sb[0:64, pr * T + qb * 512: pr * T + (qb + 1) * 512],
                        unA[0:64, :], bcA[:])
                    stB = nrm_pool.tile([64, 512], BF16, name=f"stB{pr}{qb}", tag="stB")
                    nc.vector.tensor_mul(stB[:], unB[0:64, :], bcB[:])
                    nc.sync.dma_start(
                        yT_sb[64:128, pr * T + qb * 512: pr * T + (qb + 1) * 512],
                        stB[:])

                    # ---- proj, interleaved: once pair 3 normalizes query
                    # block qb, the output rows t in [4qb, 4qb+4) have all four
                    # pairs' yT ready — emit them here so the projection hides
                    # under pair 3's remaining (ACT-bound) attention instead of
                    # serializing at the kernel tail.
                    if pr == 3:
                        for t in range(4 * qb, 4 * qb + 4):
                            for n in range(2):
                                mm = ps_mm.tile([P, 512], F32, name=f"mmo{t}{n}",
                                                tag="mm")
                                for kp in range(4):
                                    nc.tensor.matmul(
                                        mm[:],
                                        yT_sb[:, kp * T + t * P: kp * T + (t + 1) * P],
                                        wp_sb[:, kp * D + n * 512:
                                              kp * D + (n + 1) * 512],
                                        start=(kp == 0), stop=(kp == 3),
                                    )
                                ost = nrm_pool.tile([P, 512], F32,
                                                    name=f"ost{t}{n}", tag="ost")
                                nc.scalar.activation(
                                    ost[:], mm[:],
                                    mybir.ActivationFunctionType.Copy)
                                nc.sync.dma_start(
                                    out[t * P:(t + 1) * P, n * 512:(n + 1) * 512],
                                    ost[:])

    nc.compile()
    return nc


def _host_masks():
    k = np.arange(P)[:, None]
    q = np.arange(P)[None, :]
    return (k <= q).astype(ml_dtypes.bfloat16)


def _in_maps(x, w_qkv, w_proj):
    masks = _host_masks()
    maps = []
    for c in range(N_CORES):
        b, g = c // 2, c % 2
        xTc = np.ascontiguousarray(x[b].T.astype(np.float32))
        wqk = np.concatenate(
            [w_qkv[:, g * DL:(g + 1) * DL], w_qkv[:, D + g * DL: D + (g + 1) * DL]],
            axis=1).astype(np.float32)
        wvc = np.ascontiguousarray(w_qkv[:, 2 * D + g * DL: 2 * D + (g + 1) * DL]
                                   .astype(np.float32))
        wpc = np.ascontiguousarray(w_proj[g * DL:(g + 1) * DL, :]).astype(
            ml_dtypes.bfloat16)
        maps.append({"xT": xTc, "wqk": wqk, "wv": wvc, "wp": wpc, "masks": masks})
    return maps


def kernel(x, w_qkv, w_proj):
    x = np.asarray(x, dtype=np.float32)
    w_qkv = np.asarray(w_qkv, dtype=np.float32)
    w_proj = np.asarray(w_proj, dtype=np.float32)

    if "nc" not in _CACHE:
        _CACHE["nc"] = _build_program()
    nc = _CACHE["nc"]

    tmpdir = tempfile.mkdtemp(prefix="attn_neff_") if PROFILE else None
    res = bass_utils.run_bass_kernel_spmd(
        nc, _in_maps(x, w_qkv, w_proj), core_ids=list(range(N_CORES)),
        trace=PROFILE, tmpdir=tmpdir,
    )
    _CACHE["last_result"] = res

    out = np.empty((B, T, D), dtype=np.float32)
    for b in range(B):
        out[b] = res.results[2 * b]["out"] + res.results[2 * b + 1]["out"]
    return out
